# revision 32
# baseline (speedup 1.0000x reference)
"""LoRA MultiheadAttention on 8 Trainium2 NeuronCores (Bass/Tile).

Sharding: core c = (batch n = c//2, head-group hg = c%2); each core handles
6 of 12 heads for one of 4 batches. LoRA is folded into the projection
weights on the host (W_eff = W + scale * up @ down — exact identity).
Inputs ship pre-transposed (E-major) and pre-converted to f16 on the host.

Per-core kernel (all-f16 matmuls; fp8 DoubleRow was tried and is slower
on real HW for these shapes):
  - q/k/v projections in f16 (k needs no bias: a score term constant
    over the softmax axis cancels; v bias folds into the output bias
    host-side)
  - full softmax with the denominator via an appended ones column in v
    (attnV produces [64 o-rows | denom row] per head, in f16); attnV for
    head h interleaves with scores for head h+1 so the PE never drains
  - exp splits across engines: Scalar does native exp(), Vector does a
    Schraudolph f16 approximation (int16 bitcast) on 13/32 of the score
    chunks so neither engine gates the Tensor engine
  - normalize runs straight out of PSUM per 512-wide l-chunk: 1/denom
    via exp(-ln(D)) on Scalar (one act table covers exp+ln+copy),
    partition-broadcast on GpSimd, multiply on Vector
  - out-projection is interleaved with the last head's attnV at l-chunk
    granularity; partials stream to DRAM as f16
The host sums the two partial out-projections per batch and adds the
total output bias (pure unshard glue).
"""
import numpy as np

import concourse.tile as tile
from concourse import bacc, mybir
from concourse.bass_utils import run_bass_kernel_spmd

L, N, E, H, R = 2048, 4, 768, 12, 16
ALPHA = 16.0
LORA_SCALE = ALPHA / R
HD = E // H          # 64
HG = 2               # head groups (column-parallel dimension)
HPG = H // HG        # 6 heads per group
EG = E // HG         # 384 columns per group
NC_ = 8
F32 = mybir.dt.float32
F16 = mybir.dt.float16
I16 = mybir.dt.int16
SCALE = 1.0 / float(np.sqrt(HD))  # folded into exp's input scale

# Schraudolph f16 exp: i16 = round(A*(SCALE*x) + B); bitcast i16 -> f16.
EXP_A = 1024.0 / float(np.log(2.0)) * SCALE
EXP_B = 15360.0 - 44.0

KC = E // 128    # 6 contraction chunks
EC = EG // 128   # 3 output chunks per projection
LT = L // 128    # 16 s tiles
VW = HPG * (HD + 1)  # 390: per-head 64 v cols + 1 ones col

_CACHED = {}


class _Bacc(bacc.Bacc):
    """Pin all activations to the natural_log_exp_and_others table (it
    contains exp, ln, and copy — the only funcs this kernel uses). The
    default per-func first-match choice alternates between the exp-only
    and ln-only tables, costing a 1.3us ACT_TABLE_LOAD per transition."""

    def insert_act_table_loads(self):
        import bass_rust as _br
        from concourse.hw_specs import get_activation_tables

        has_activation = any(
            isinstance(i, mybir.InstActivation)
            for b in self.main_func.blocks
            for i in b.instructions
        )
        if not has_activation:
            return
        keep = "natural_log_exp_and_others"
        tables = [
            (nm, (fs if nm == keep else set()))
            for nm, fs in get_activation_tables(self.m.arch).items()
        ]
        _br.insert_act_table_loads(self, tables)


def _build():
    nc = _Bacc()
    xqT = nc.dram_tensor("xqT", [E, L], F16, kind="ExternalInput")
    xkT = nc.dram_tensor("xkT", [E, L], F16, kind="ExternalInput")
    xvT = nc.dram_tensor("xvT", [E, L], F16, kind="ExternalInput")
    wqT = nc.dram_tensor("wqT", [E, EG], F16, kind="ExternalInput")
    wkT = nc.dram_tensor("wkT", [E, EG], F16, kind="ExternalInput")
    wvT = nc.dram_tensor("wvT", [E, EG], F16, kind="ExternalInput")
    woT = nc.dram_tensor("woT", [EG, E], F16, kind="ExternalInput")
    bq = nc.dram_tensor("bq", [EG], F32, kind="ExternalInput")
    out = nc.dram_tensor("out", [E, L], F16, kind="ExternalOutput")

    Exp = mybir.ActivationFunctionType.Exp
    Ln = mybir.ActivationFunctionType.Ln
    Copy = mybir.ActivationFunctionType.Copy

    with tile.TileContext(nc) as tc:
        with (
            tc.tile_pool(name="persist", bufs=1) as persist,
            tc.tile_pool(name="psum", bufs=1, space="PSUM") as psum,
        ):
            # ---- persistent tiles ----
            wo16 = []
            for j in range(EC):
                wt = persist.tile([128, E], F16, name=f"wo16_{j}")
                nc.scalar.dma_start(wt[:], woT[j * 128:(j + 1) * 128, :])
                wo16.append(wt)
            bias_q = []
            for j in range(EC):
                bt = persist.tile([128, 1], F32, name=f"bq{j}")
                nc.scalar.dma_start(bt[:], bq[j * 128:(j + 1) * 128])
                bias_q.append(bt)
            qkT = {}
            for pname in ("q", "k"):
                for e in range(EC):
                    qkT[pname, e] = persist.tile([128, L], F16,
                                                 name=f"{pname}T{e}")
            v_aug = [persist.tile([128, VW], F16, name=f"v_aug{st}")
                     for st in range(LT)]
            oT = [persist.tile([128, L], F16, name=f"oT{j}")
                  for j in range(EC)]

            # PE warm-up: the first input DMAs land ~10us after the ~7us
            # engine preamble; dummy matmuls on a memset tile fill that
            # window and ramp the PE p-state so the projections start at
            # full clock instead of trickling in at the cold rate.
            warm = persist.tile([128, 512], F16, name="warm")
            nc.gpsimd.memset(warm[:], 0.0)
            for i in range(12):
                wps = psum.tile([128, 512], F32, tag="ot", bufs=2,
                                name="warmps")
                nc.tensor.matmul(wps[:], warm[:, 0:128], warm[:],
                                 start=True, stop=True)



            # ---- phase 1: projections (w/x staging dies with this pool) ----
            with tc.tile_pool(name="wx", bufs=1) as wx:
                x16 = {}
                w16 = {}
                # x DMAs: 512-col slices, slice-major, issued from SP; the
                # slice split spreads one tensor over many DMA queues (a
                # whole [128, L] chunk on one queue is a ~23us lead-in) and
                # slice-major order gives every kk-chunk its first slice
                # early so the first accumulation groups start fast.
                # Weights issue from the ACT sequencer in parallel — the SP
                # sequencer alone takes ~565ns per DMA and becomes the
                # bottleneck if it issues everything.
                for pname, xdram, wdram in (("q", xqT, wqT), ("k", xkT, wkT),
                                            ("v", xvT, wvT)):
                    for j in range(KC):
                        x16[pname, j] = wx.tile([128, L], F16, tag="x",
                                                bufs=18, name="x16")
                        w16[pname, j] = wx.tile([128, EG], F16, tag="w",
                                                bufs=18, name="w16")
                        nc.scalar.dma_start(
                            w16[pname, j][:], wdram[j * 128:(j + 1) * 128, :])
                    for s4 in range(4):
                        csl = slice(s4 * 512, (s4 + 1) * 512)
                        for j in range(KC):
                            nc.sync.dma_start(
                                x16[pname, j][:, csl],
                                xdram[j * 128:(j + 1) * 128, csl])

                for pname in ("q", "k"):
                    for e in range(EC):
                        dst = qkT[pname, e]
                        for lc in range(2):
                            mm = psum.tile([128, 1024], F32, tag="sc", bufs=3,
                                           name="mm_proj")
                            for half in range(2):
                                o_sl = mm[:, half * 512:(half + 1) * 512]
                                l0 = lc * 1024 + half * 512
                                for kk in range(KC):
                                    nc.tensor.matmul(
                                        o_sl,
                                        w16[pname, kk][:, e * 128:(e + 1) * 128],
                                        x16[pname, kk][:, l0:l0 + 512],
                                        start=(kk == 0), stop=(kk == KC - 1),
                                    )
                            d_sl = dst[:, lc * 1024:(lc + 1) * 1024]
                            if pname == "q":
                                nc.vector.tensor_scalar_add(
                                    d_sl, mm[:], bias_q[e][:])
                            else:
                                nc.scalar.activation(d_sl, mm[:], Copy)
                for st in range(LT):
                    mm = psum.tile([128, 1024], F32, tag="sc", bufs=3,
                                   name="mm_vproj")
                    for kk in range(KC):
                        nc.tensor.matmul(
                            mm[:, 0:EG],
                            x16["v", kk][:, st * 128:(st + 1) * 128],
                            w16["v", kk][:],
                            start=(kk == 0), stop=(kk == KC - 1),
                        )
                    vt = v_aug[st]
                    grp = vt.rearrange("p (h c) -> p h c", c=HD + 1)
                    nc.vector.tensor_copy(
                        grp[:, :, 0:HD],
                        mm[:, 0:EG].rearrange("p (h c) -> p h c", c=HD),
                    )
                    nc.vector.memset(grp[:, :, HD:HD + 1], 1.0)

            # ---- phase 2: attention ----
            with (
                tc.tile_pool(name="attn", bufs=1) as attnp,
                tc.tile_pool(name="small", bufs=1) as small,
            ):
                attn = {}   # (h, st) -> [128, L] f16 tile

                def emit_scores(h, st_lo, st_hi):
                    et, pb = h // 2, (h % 2) * 64
                    qs = qkT["q", et][pb:pb + 64, :]
                    ks = qkT["k", et][pb:pb + 64, :]
                    for st in range(st_lo, st_hi):
                        at = attnp.tile([128, L], F16, tag="attn", bufs=28,
                                        name="attn")
                        attn[h, st] = at
                        for lc in range(2):
                            sc = psum.tile([128, 1024], F32, tag="sc", bufs=3,
                                           name="mm_sc")
                            for half in range(2):
                                l0 = lc * 1024 + half * 512
                                nc.tensor.matmul(
                                    sc[:, half * 512:(half + 1) * 512],
                                    ks[:, st * 128:(st + 1) * 128],
                                    qs[:, l0:l0 + 512],
                                    start=True, stop=True,
                                )
                            a_sl = at[:, lc * 1024:(lc + 1) * 1024]
                            if (st * 2 + lc) % 5 in (1, 3):
                                nc.vector.tensor_scalar(
                                    a_sl.bitcast(I16), sc[:], EXP_A, EXP_B,
                                    mybir.AluOpType.mult,
                                    mybir.AluOpType.add,
                                )
                            else:
                                nc.scalar.activation(a_sl, sc[:], Exp,
                                                     scale=SCALE)

                def emit_attnv_lc(h, lc4):
                    ot = psum.tile([65, 512], F32, tag="ot", bufs=2,
                                   name="ot")
                    for st in range(LT):
                        nc.tensor.matmul(
                            ot[:],
                            v_aug[st][:, h * (HD + 1):(h + 1) * (HD + 1)],
                            attn[h, st][:, lc4 * 512:(lc4 + 1) * 512],
                            start=(st == 0), stop=(st == LT - 1),
                        )
                    return ot

                def emit_norm_chunk(h, ot, lc4):
                    # normalize straight out of PSUM: ln/exp give 1/D from
                    # the ones-column row, the o rows never stage to SBUF
                    et, pb = h // 2, (h % 2) * 64
                    sl = slice(lc4 * 512, (lc4 + 1) * 512)
                    t = small.tile([1, 512], F32, tag="t", bufs=2, name="lnD")
                    nc.scalar.activation(t[:], ot[64:65, :], Ln)
                    rec = small.tile([1, 512], F16, tag="rec", bufs=2,
                                     name="rec")
                    nc.scalar.activation(rec[:], t[:], Exp, scale=-1.0)
                    rbc = small.tile([64, 512], F16, tag="rbc", bufs=2,
                                     name="rbc")
                    nc.gpsimd.partition_broadcast(rbc[:], rec[:])
                    nc.vector.tensor_mul(
                        oT[et][pb:pb + 64, sl], ot[0:64, :], rbc[:])

                def emit_outproj_lc(lc4):
                    for eo in range(6):
                        po = psum.tile([128, 1024], F32, tag="sc", bufs=3,
                                       name="po")
                        for j in range(EC):
                            nc.tensor.matmul(
                                po[:, 0:512],
                                wo16[j][:, eo * 128:(eo + 1) * 128],
                                oT[j][:, lc4 * 512:(lc4 + 1) * 512],
                                start=(j == 0), stop=(j == EC - 1),
                            )
                        osb = small.tile([128, 512], F16, tag="osb", bufs=4,
                                         name="osb")
                        if eo % 2 == 0:
                            nc.scalar.activation(osb[:], po[:, 0:512], Copy)
                        else:
                            nc.vector.tensor_copy(osb[:], po[:, 0:512])
                        nc.sync.dma_start(
                            out[eo * 128:(eo + 1) * 128,
                                lc4 * 512:(lc4 + 1) * 512],
                            osb[:])

                emit_scores(0, 0, LT)
                for h in range(HPG):
                    if h + 1 < HPG:
                        for lc4 in range(4):
                            ot = emit_attnv_lc(h, lc4)
                            emit_scores(h + 1, lc4 * 4, (lc4 + 1) * 4)
                            emit_norm_chunk(h, ot, lc4)
                    else:
                        # last head: interleave out-projection per l-chunk,
                        # one chunk behind so the normalize chain hides
                        # under the next attnV scan
                        for lc4 in range(4):
                            ot = emit_attnv_lc(h, lc4)
                            emit_norm_chunk(h, ot, lc4)
                            if lc4 > 0:
                                emit_outproj_lc(lc4 - 1)
                        emit_outproj_lc(3)
    nc.finalize()
    return nc


def kernel(query, key, value, in_proj_weight, in_proj_bias,
           q_down, q_up, k_down, k_up, v_down, v_up,
           out_proj_weight, out_proj_bias, out_down, out_up):
    if "nc" not in _CACHED:
        _CACHED["nc"] = _build()
    nc = _CACHED["nc"]

    f = np.float32
    h16 = np.float16
    # fold LoRA into the projection weights (exact algebraic identity)
    w_eff = {}
    for i, (dn, up) in enumerate(((q_down, q_up), (k_down, k_up),
                                  (v_down, v_up))):
        w = in_proj_weight[i * E:(i + 1) * E].astype(f)
        w_eff[i] = w + LORA_SCALE * (up.astype(f) @ dn.astype(f))
    wo_eff = out_proj_weight.astype(f) + LORA_SCALE * (
        out_up.astype(f) @ out_down.astype(f))

    in_maps = []
    for c in range(NC_):
        n, hg = c // 2, c % 2
        sl = slice(hg * EG, (hg + 1) * EG)
        m = {
            "xqT": np.ascontiguousarray(query[:, n, :].T, dtype=h16),
            "xkT": np.ascontiguousarray(key[:, n, :].T, dtype=h16),
            "xvT": np.ascontiguousarray(value[:, n, :].T, dtype=h16),
            "wqT": np.ascontiguousarray(w_eff[0][sl].T, dtype=h16),
            "wkT": np.ascontiguousarray(w_eff[1][sl].T, dtype=h16),
            "wvT": np.ascontiguousarray(w_eff[2][sl].T, dtype=h16),
            "woT": np.ascontiguousarray(wo_eff[:, sl].T, dtype=h16),
            "bq": np.ascontiguousarray(in_proj_bias[0:E][sl], dtype=f),
        }
        in_maps.append(m)

    _CACHED["in_maps"] = in_maps
    res = run_bass_kernel_spmd(nc, in_maps, list(range(NC_)))
    outp = np.empty((L, N, E), dtype=np.float32)
    bo_total = out_proj_bias.astype(f) + wo_eff @ np.ascontiguousarray(
        in_proj_bias[2 * E:3 * E], dtype=f)
    for n in range(N):
        outp[:, n, :] = (res.results[2 * n]["out"].astype(f)
                         + res.results[2 * n + 1]["out"].astype(f)).T + bo_total
    return outp
